# revision 11
# baseline (speedup 1.0000x reference)
"""NT-Xent loss kernel for 8x Trainium2 NeuronCores (Bass/Tile).

Math: z = concat(z_i, z_j) [8192, 256]; zn = z / ||z||_row;
sim = (zn @ zn.T) / 0.5. Since rows are unit-norm, diag(sim) == 2.0 and all
logits lie in [-2, 2], so no max-subtraction pass is needed:
  lse_r = log(sum_j exp(sim_rj - 2) - 1) + 2      (the -1 removes the diag)
  pos_r = 2 * (zn_r . zn_partner)                 partner(r) = (r + 4096) % 8192
  loss  = mean(lse - pos)

Sharding: core k owns rows [1024k, 1024k+1024) of the sim matrix and computes
(lse - pos) for those rows against the full zn (replicated). The host sums the
8 per-core [128, 8] shards (the scalar all-reduce step) and divides by N.

Engine plan per core (engines are in-order; emission is software-pipelined
with one-slab lookahead so no queue stalls cascade):
  sync   - 6 input DMAs (HWDGE, f32, 16KB contiguous per partition) + output
  gpsimd - f32 -> bf16 casts (otherwise idle)
  vector - row norms (mul+reduce), normalize scales, PSUM->SBUF copies, pos
  tensor - PE transposes (bf16, via identity) + the 1024x8192 bf16 matmul
  scalar - one table load (patched), exp with fused row-sum accum, ln
"""

import os
import numpy as np

B = 4096
N = 8192
D = 256
P = 128
NCORES = 8
RPC = 1024  # rows per core
RT = RPC // P  # 8 row tiles per core
NU = 4  # full-z slabs == column blocks of the sim matrix
TPU = 16  # row tiles per slab
CHUNK = 2048  # PSUM chunk (4 banks) == one slab's columns
HS = N // CHUNK  # 4 column chunks per row tile (== NU)

_cache: dict = {}
LAST_EXEC_TIME_NS = None
LAST_RESULTS = None


def _patch_act_tables():
    """The act-table insertion pass greedily picks the first set containing
    each activation function, which bounces between `natural_log` and
    `exp_and_others` (one ~2.7us table load per switch). Strip ln/exp/copy
    from every set except the combined `natural_log_exp_and_others` (indices
    are preserved, so the correct table is still referenced) -> one load."""
    import concourse.hw_specs as hw_specs
    import concourse.bacc as bacc
    import concourse.mybir as mybir

    if getattr(hw_specs, "_ntx_patched", False):
        return
    orig = hw_specs.get_activation_tables

    def patched(module_arch):
        tabs = orig(module_arch)
        strip = {
            mybir.ActivationFunctionType.Ln,
            mybir.ActivationFunctionType.Exp,
            mybir.ActivationFunctionType.Copy,
            mybir.ActivationFunctionType.Identity,
        }
        for name, fns in tabs.items():
            if name != "natural_log_exp_and_others":
                fns -= strip
        return tabs

    hw_specs.get_activation_tables = patched
    bacc.get_activation_tables = patched
    hw_specs._ntx_patched = True


def _build_nc():
    import concourse.bacc as bacc
    import concourse.tile as tile
    import concourse.mybir as mybir
    from concourse.masks import make_identity
    from contextlib import ExitStack

    _patch_act_tables()

    f32 = mybir.dt.float32
    bf16 = mybir.dt.bfloat16
    Exp = mybir.ActivationFunctionType.Exp
    Ln = mybir.ActivationFunctionType.Ln
    MUL = mybir.AluOpType.mult
    ADD = mybir.AluOpType.add
    AxX = mybir.AxisListType.X

    nc = bacc.Bacc("TRN2", target_bir_lowering=False, debug=False, num_devices=NCORES)
    zall = nc.dram_tensor("zall", [N, D], f32, kind="ExternalInput").ap()
    zrows = nc.dram_tensor("zrows", [RPC, D], f32, kind="ExternalInput").ap()
    zpart = nc.dram_tensor("zpart", [RPC, D], f32, kind="ExternalInput").ap()
    out = nc.dram_tensor("out", [P, RT], f32, kind="ExternalOutput").ap()

    with tile.TileContext(nc) as tc, ExitStack() as ctx:
        const_pool = ctx.enter_context(tc.tile_pool(name="const", bufs=1))
        ident = const_pool.tile([P, P], bf16)
        make_identity(nc, ident[:])
        bias_m2 = const_pool.tile([P, 1], f32)
        nc.gpsimd.memset(bias_m2[:], -2.0)

        zpool = ctx.enter_context(tc.tile_pool(name="z", bufs=1))
        zu = [
            zpool.tile([P, TPU * D], bf16, tag=f"zu{u}", name=f"zu{u}")
            for u in range(NU)
        ]
        zr = zpool.tile([P, RT * D], bf16, tag="zr")
        zp = zpool.tile([P, RT * D], bf16, tag="zp")

        stats = ctx.enter_context(tc.tile_pool(name="stats", bufs=1))
        ss = stats.tile([P, 80], f32)
        lns = stats.tile([P, 80], f32)
        invn = stats.tile([P, 80], f32)
        dots = stats.tile([P, RT], f32)
        posM = stats.tile([P, RT], f32)
        sums = stats.tile([P, RT * HS], f32)
        Ssum = stats.tile([P, RT], f32)
        lnS = stats.tile([P, RT], f32)
        outsb = stats.tile([P, RT], f32)

        scr_pool = ctx.enter_context(tc.tile_pool(name="scr", bufs=2))
        fpool = ctx.enter_context(tc.tile_pool(name="f32stage", bufs=2))
        tpool = ctx.enter_context(tc.tile_pool(name="zt", bufs=1))
        znT = [
            tpool.tile([P, N], bf16, tag=f"znT{k}", name=f"znT{k}") for k in range(2)
        ]
        znrT = [
            tpool.tile([P, RPC], bf16, tag=f"znrT{k}", name=f"znrT{k}")
            for k in range(2)
        ]
        ps_pool = ctx.enter_context(tc.tile_pool(name="ps", bufs=2, space="PSUM"))

        # ---- input DMAs (all issued up front; sync FIFO drains at full BW).
        # Layout: partition p holds 8/16 consecutive rows (16KB contiguous
        # DRAM read per partition -> large descriptors). "Row tile" j of a
        # slab = column slice [:, jD:(j+1)D]; the row relabeling is harmless:
        # rhs covers every row exactly once and the output is summed.
        stg = {}

        def load(name, src2d, cols):
            t = fpool.tile([P, cols * D], f32, tag="zf", name=f"stg_{name}")
            nc.sync.dma_start(
                out=t[:].rearrange("p (n d) -> p n d", d=D),
                in_=src2d.rearrange("(p n) d -> p n d", p=P),
            )
            stg[name] = t

        load("zr", zrows, RT)
        load("zp", zpart, RT)
        for u in range(NU):
            load(f"u{u}", zall[2048 * u : 2048 * (u + 1), :], TPU)

        # ---- casts on gpsimd (idle engine; in-order behind each DMA)
        nc.gpsimd.tensor_copy(zr[:], stg["zr"][:])
        nc.gpsimd.tensor_copy(zp[:], stg["zp"][:])
        for u in range(NU):
            nc.gpsimd.tensor_copy(zu[u][:], stg[f"u{u}"][:])

        # ---- per-block front-end: norms -> invn -> normalize in place
        def front(dst, col0, ntiles, normalize=True):
            sq = scr_pool.tile([P, ntiles * D], bf16, tag="sq", name="sq")
            nc.vector.tensor_mul(sq[:], dst[:], dst[:])
            nc.vector.tensor_reduce(
                out=ss[:, col0 : col0 + ntiles],
                in_=sq[:].rearrange("p (n d) -> p n d", d=D),
                axis=AxX,
                op=ADD,
            )
            nc.scalar.activation(
                lns[:, col0 : col0 + ntiles], ss[:, col0 : col0 + ntiles], Ln
            )
            nc.scalar.activation(
                invn[:, col0 : col0 + ntiles],
                lns[:, col0 : col0 + ntiles],
                Exp,
                bias=0.0,
                scale=-0.5,
            )
            if normalize:
                for j in range(ntiles):
                    sl = dst[:, j * D : (j + 1) * D]
                    nc.vector.tensor_scalar_mul(
                        sl, sl, invn[:, col0 + j : col0 + j + 1]
                    )

        front(zr, 64, RT)
        front(zp, 72, RT, normalize=False)

        # pos = 2 * (zn_r . z_p) / ||z_p||   (zp stays unnormalized)
        dg = scr_pool.tile([P, RT * D], bf16, tag="sq", name="dg")
        nc.vector.tensor_mul(dg[:], zr[:], zp[:])
        nc.vector.tensor_reduce(
            out=dots[:],
            in_=dg[:].rearrange("p (n d) -> p n d", d=D),
            axis=AxX,
            op=ADD,
        )
        for j in range(RT):
            nc.vector.tensor_scalar(
                out=posM[:, j : j + 1],
                in0=dots[:, j : j + 1],
                scalar1=invn[:, 72 + j : 73 + j],
                scalar2=2.0,
                op0=MUL,
                op1=MUL,
            )

        # ---- own-block transposes -> znrT[k] [128(d-half), 1024(rows)]
        for k in range(2):
            for c2 in range(2):
                pt = ps_pool.tile([P, 512], bf16, tag="ps", name="pt")
                for b in range(4):
                    j = 4 * c2 + b
                    nc.tensor.transpose(
                        pt[:, b * P : (b + 1) * P],
                        zr[:, j * D + k * P : j * D + k * P + P],
                        ident[:],
                    )
                nc.vector.tensor_copy(znrT[k][:, c2 * 512 : (c2 + 1) * 512], pt[:])

        # ---- pipelined main: front(u) runs one column-block ahead
        front(zu[0], 0, TPU)
        front(zu[1], TPU, TPU)

        for h in range(HS):
            # transposes of slab h -> znT columns [2048h, 2048h+2048)
            for k in range(2):
                for q in range(4):
                    pt = ps_pool.tile([P, 512], bf16, tag="ps", name="pt")
                    for b in range(4):
                        j = 4 * q + b
                        nc.tensor.transpose(
                            pt[:, b * P : (b + 1) * P],
                            zu[h][:, j * D + k * P : j * D + k * P + P],
                            ident[:],
                        )
                    nc.vector.tensor_copy(
                        znT[k][:, h * CHUNK + q * 512 : h * CHUNK + (q + 1) * 512],
                        pt[:],
                    )
            # one-ahead front for the slab after next
            if h + 2 < NU:
                front(zu[h + 2], TPU * (h + 2), TPU)
            # matmuls + exp row-sums for all row tiles against this block
            for t in range(RT):
                ps = ps_pool.tile([P, CHUNK], f32, tag="ps", name="ps")
                for k in range(2):
                    for q in range(4):
                        nc.tensor.matmul(
                            ps[:, q * 512 : (q + 1) * 512],
                            znrT[k][:, t * P : (t + 1) * P],
                            znT[k][:, h * CHUNK + q * 512 : h * CHUNK + (q + 1) * 512],
                            start=(k == 0),
                            stop=(k == 1),
                            skip_group_check=True,
                        )
                es = scr_pool.tile([P, CHUNK], bf16, tag="escr", name="es")
                idx = HS * t + h
                nc.scalar.activation(
                    es[:],
                    ps[:],
                    Exp,
                    bias=bias_m2[:],
                    scale=2.0,
                    accum_out=sums[:, idx : idx + 1],
                )

        # ---- lse - pos = ln(S - 1) + 2 - pos
        nc.vector.tensor_reduce(
            out=Ssum[:],
            in_=sums[:].rearrange("p (t h) -> p t h", h=HS),
            axis=AxX,
            op=ADD,
        )
        nc.vector.tensor_scalar_add(Ssum[:], Ssum[:], -1.0)
        nc.scalar.activation(lnS[:], Ssum[:], Ln)
        nc.vector.tensor_sub(outsb[:], lnS[:], posM[:])
        nc.vector.tensor_scalar_add(outsb[:], outsb[:], 2.0)
        nc.sync.dma_start(out=out[:], in_=outsb[:])

    nc.compile()
    return nc


def _install_ntff_hook():
    """Provide antenv.axon_hooks (absent in this image) so trace=True can
    capture NTFF profiles via libaxon_pjrt's C ABI."""
    import sys, types, ctypes, contextlib

    try:
        from antenv.axon_hooks import get_axon_ntff_profile_hook  # noqa: F401

        return True
    except ImportError:
        pass
    try:
        import antenv

        lib = ctypes.CDLL("/opt/axon/libaxon_pjrt.so")
        if not hasattr(lib, "axon_start_nrt_profile"):
            return False
        lib.axon_start_nrt_profile.argtypes = [
            ctypes.POINTER(ctypes.c_int64),
            ctypes.c_size_t,
        ]
        lib.axon_start_nrt_profile.restype = ctypes.c_int64
        lib.axon_stop_nrt_profile.argtypes = [ctypes.c_char_p]
        lib.axon_stop_nrt_profile.restype = ctypes.c_int64

        @contextlib.contextmanager
        def _hook(output_dir, device_ids):
            import jax

            jax.devices()
            if device_ids:
                ids = (ctypes.c_int64 * len(device_ids))(*device_ids)
                rc = lib.axon_start_nrt_profile(ids, len(device_ids))
            else:
                rc = lib.axon_start_nrt_profile(None, 0)
            if rc != 0:
                raise RuntimeError(f"axon_start_nrt_profile rc={rc}")
            try:
                yield
            finally:
                n = lib.axon_stop_nrt_profile(str(output_dir).encode())
                print(f"ntff profile: {n} file(s) written to {output_dir}")

        mod = types.ModuleType("antenv.axon_hooks")
        _state = {"hook": _hook}
        mod.set_axon_ntff_profile_hook = lambda h: _state.__setitem__("hook", h)
        mod.get_axon_ntff_profile_hook = lambda: _state["hook"]
        sys.modules["antenv.axon_hooks"] = mod
        antenv.axon_hooks = mod
        return True
    except Exception as e:
        print(f"ntff hook install failed: {e}")
        return False


def _get_nc():
    if "nc" not in _cache:
        _cache["nc"] = _build_nc()
    return _cache["nc"]


def kernel(z_i, z_j):
    global LAST_EXEC_TIME_NS, LAST_RESULTS
    from concourse.bass_utils import run_bass_kernel_spmd

    z = np.ascontiguousarray(
        np.concatenate([np.asarray(z_i), np.asarray(z_j)], axis=0), dtype=np.float32
    )
    in_maps = []
    for k in range(NCORES):
        lo = RPC * k
        plo = (lo + B) % N
        in_maps.append(
            {
                "zall": z,
                "zrows": np.ascontiguousarray(z[lo : lo + RPC]),
                "zpart": np.ascontiguousarray(z[plo : plo + RPC]),
            }
        )

    nc = _get_nc()
    trace = os.environ.get("BASS_KERNEL_TRACE", "0") == "1"
    if trace:
        trace = _install_ntff_hook()
    res = run_bass_kernel_spmd(nc, in_maps, core_ids=list(range(NCORES)), trace=trace)
    LAST_RESULTS = res
    LAST_EXEC_TIME_NS = res.exec_time_ns

    total = 0.0
    for k in range(NCORES):
        total += float(np.sum(np.asarray(res.results[k]["out"], dtype=np.float64)))
    return np.array(total / N, dtype=np.float32)


# revision 12
# speedup vs baseline: 1.5498x; 1.5498x over previous
"""NT-Xent loss kernel for 8x Trainium2 NeuronCores (Bass/Tile).

Math: z = concat(z_i, z_j) [8192, 256]; zn = z / ||z||_row;
sim = (zn @ zn.T) / 0.5. Since rows are unit-norm, diag(sim) == 2.0 and all
logits lie in [-2, 2], so no max-subtraction pass is needed:
  lse_r = log(sum_j exp(sim_rj - 2) - 1) + 2      (the -1 removes the diag)
  pos_r = 2 * (zn_r . zn_partner)                 partner(r) = (r + 4096) % 8192
  loss  = mean(lse - pos)

Sharding: core k owns rows [1024k, 1024k+1024) of the sim matrix and computes
(lse - pos) for those rows against the full zn (replicated). The host sums the
8 per-core [128, 8] shards (the scalar all-reduce step) and divides by N.

Engine plan per core (engines are in-order; emission is software-pipelined
with one-slab lookahead so no queue stalls cascade):
  sync   - 6 input DMAs (HWDGE, f32, 16KB contiguous per partition) + output
  gpsimd - f32 -> bf16 casts (otherwise idle)
  vector - row norms (mul+reduce), normalize scales, PSUM->SBUF copies, pos
  tensor - PE transposes (bf16, via identity) + the 1024x8192 bf16 matmul
  scalar - one table load (patched), exp with fused row-sum accum, ln
"""

import os
import numpy as np

B = 4096
N = 8192
D = 256
P = 128
NCORES = 8
RPC = 1024  # rows per core
RT = RPC // P  # 8 row tiles per core
NU = 4  # full-z slabs == column blocks of the sim matrix
TPU = 16  # row tiles per slab
CHUNK = 2048  # PSUM chunk (4 banks) == one slab's columns
HS = N // CHUNK  # 4 column chunks per row tile (== NU)

_cache: dict = {}
LAST_EXEC_TIME_NS = None
LAST_RESULTS = None


def _patch_act_tables():
    """The act-table insertion pass greedily picks the first set containing
    each activation function, which bounces between `natural_log` and
    `exp_and_others` (one ~2.7us table load per switch). Strip ln/exp/copy
    from every set except the combined `natural_log_exp_and_others` (indices
    are preserved, so the correct table is still referenced) -> one load."""
    import concourse.hw_specs as hw_specs
    import concourse.bacc as bacc
    import concourse.mybir as mybir

    if getattr(hw_specs, "_ntx_patched", False):
        return
    orig = hw_specs.get_activation_tables

    def patched(module_arch):
        tabs = orig(module_arch)
        strip = {
            mybir.ActivationFunctionType.Ln,
            mybir.ActivationFunctionType.Exp,
            mybir.ActivationFunctionType.Copy,
            mybir.ActivationFunctionType.Identity,
        }
        for name, fns in tabs.items():
            if name != "natural_log_exp_and_others":
                fns -= strip
        return tabs

    hw_specs.get_activation_tables = patched
    bacc.get_activation_tables = patched
    hw_specs._ntx_patched = True


def _build_nc():
    import concourse.bacc as bacc
    import concourse.tile as tile
    import concourse.mybir as mybir
    from concourse.masks import make_identity
    from contextlib import ExitStack

    _patch_act_tables()

    f32 = mybir.dt.float32
    bf16 = mybir.dt.bfloat16
    Exp = mybir.ActivationFunctionType.Exp
    Ln = mybir.ActivationFunctionType.Ln
    MUL = mybir.AluOpType.mult
    ADD = mybir.AluOpType.add
    AxX = mybir.AxisListType.X

    nc = bacc.Bacc("TRN2", target_bir_lowering=False, debug=False, num_devices=NCORES)
    zall = nc.dram_tensor("zall", [N, D], f32, kind="ExternalInput").ap()
    zrows = nc.dram_tensor("zrows", [RPC, D], f32, kind="ExternalInput").ap()
    zpart = nc.dram_tensor("zpart", [RPC, D], f32, kind="ExternalInput").ap()
    out = nc.dram_tensor("out", [P, RT], f32, kind="ExternalOutput").ap()

    with tile.TileContext(nc) as tc, ExitStack() as ctx:
        const_pool = ctx.enter_context(tc.tile_pool(name="const", bufs=1))
        bias_m2 = const_pool.tile([P, 1], f32)
        nc.gpsimd.memset(bias_m2[:], -2.0)

        zpool = ctx.enter_context(tc.tile_pool(name="z", bufs=1))
        zu = [
            zpool.tile([P, TPU * D], bf16, tag=f"zu{u}", name=f"zu{u}")
            for u in range(NU)
        ]
        zr = zpool.tile([P, RT * D], bf16, tag="zr")
        zp = zpool.tile([P, RT * D], bf16, tag="zp")

        stats = ctx.enter_context(tc.tile_pool(name="stats", bufs=1))
        ss = stats.tile([P, 80], f32)
        lns = stats.tile([P, 80], f32)
        invn = stats.tile([P, 80], f32)
        dots = stats.tile([P, RT], f32)
        posM = stats.tile([P, RT], f32)
        sums = stats.tile([P, RT * HS], f32)
        Ssum = stats.tile([P, RT], f32)
        lnS = stats.tile([P, RT], f32)
        outsb = stats.tile([P, RT], f32)

        scr_pool = ctx.enter_context(tc.tile_pool(name="scr", bufs=2))
        tpool = ctx.enter_context(tc.tile_pool(name="zt", bufs=1))
        dpool = ctx.enter_context(tc.tile_pool(name="dram", bufs=1, space="DRAM"))
        znd = dpool.tile([N, D], bf16, name="znd")
        znrd = dpool.tile([RPC, D], bf16, name="znrd")
        znT = [
            tpool.tile([P, N], bf16, tag=f"znT{k}", name=f"znT{k}") for k in range(2)
        ]
        znrT = [
            tpool.tile([P, RPC], bf16, tag=f"znrT{k}", name=f"znrT{k}")
            for k in range(2)
        ]
        ps_pool = ctx.enter_context(tc.tile_pool(name="ps", bufs=2, space="PSUM"))

        # ---- input cast-DMAs (SWDGE, f32 -> bf16 in the DMA datapath).
        # Layout: partition p holds 8/16 consecutive rows (16KB contiguous
        # DRAM read per partition -> large descriptors). "Row tile" j of a
        # slab = column slice [:, jD:(j+1)D] = rows {2048u + 16p + j}; the
        # relabeling is harmless: rhs covers every row exactly once and the
        # output is summed. zr/zp first: the lhsT path needs them earliest.
        nc.gpsimd.dma_start(
            out=zr[:].rearrange("p (n d) -> p n d", d=D),
            in_=zrows.rearrange("(p n) d -> p n d", p=P),
        )
        nc.gpsimd.dma_start(
            out=zp[:].rearrange("p (n d) -> p n d", d=D),
            in_=zpart.rearrange("(p n) d -> p n d", p=P),
        )
        for u in range(NU):
            nc.gpsimd.dma_start(
                out=zu[u][:].rearrange("p (n d) -> p n d", d=D),
                in_=zall[2048 * u : 2048 * (u + 1), :].rearrange(
                    "(p n) d -> p n d", p=P
                ),
            )

        # ---- per-block front-end: norms -> invn -> normalize in place
        def front(dst, col0, ntiles, normalize=True):
            sq = scr_pool.tile([P, ntiles * D], bf16, tag="sq", name="sq")
            nc.vector.tensor_mul(sq[:], dst[:], dst[:])
            nc.vector.tensor_reduce(
                out=ss[:, col0 : col0 + ntiles],
                in_=sq[:].rearrange("p (n d) -> p n d", d=D),
                axis=AxX,
                op=ADD,
            )
            nc.scalar.activation(
                lns[:, col0 : col0 + ntiles], ss[:, col0 : col0 + ntiles], Ln
            )
            nc.scalar.activation(
                invn[:, col0 : col0 + ntiles],
                lns[:, col0 : col0 + ntiles],
                Exp,
                bias=0.0,
                scale=-0.5,
            )
            if normalize:
                for j in range(ntiles):
                    sl = dst[:, j * D : (j + 1) * D]
                    nc.vector.tensor_scalar_mul(
                        sl, sl, invn[:, col0 + j : col0 + j + 1]
                    )

        front(zr, 64, RT)
        front(zp, 72, RT, normalize=False)

        # pos = 2 * (zn_r . z_p) / ||z_p||   (zp stays unnormalized)
        dg = scr_pool.tile([P, RT * D], bf16, tag="sq", name="dg")
        nc.vector.tensor_mul(dg[:], zr[:], zp[:])
        nc.vector.tensor_reduce(
            out=dots[:],
            in_=dg[:].rearrange("p (n d) -> p n d", d=D),
            axis=AxX,
            op=ADD,
        )
        for j in range(RT):
            nc.vector.tensor_scalar(
                out=posM[:, j : j + 1],
                in0=dots[:, j : j + 1],
                scalar1=invn[:, 72 + j : 73 + j],
                scalar2=2.0,
                op0=MUL,
                op1=MUL,
            )

        # ---- own-block: store normalized zr to DRAM, transpose-load (xbar)
        nc.sync.dma_start(
            out=znrd[:].rearrange("(p n) d -> p n d", p=P),
            in_=zr[:].rearrange("p (n d) -> p n d", d=D),
        )
        for k in range(2):
            nc.sync.dma_start(
                out=znrT[k][:],
                in_=znrd[:, k * P : (k + 1) * P],
                transpose=True,
            )

        # ---- pipelined main: front(u) runs one column-block ahead
        front(zu[0], 0, TPU)
        front(zu[1], TPU, TPU)

        for h in range(HS):
            # store normalized slab h to DRAM, transpose-load its znT columns
            nc.sync.dma_start(
                out=znd[2048 * h : 2048 * (h + 1), :].rearrange(
                    "(p n) d -> p n d", p=P
                ),
                in_=zu[h][:].rearrange("p (n d) -> p n d", d=D),
            )
            for k in range(2):
                nc.sync.dma_start(
                    out=znT[k][:, h * CHUNK : (h + 1) * CHUNK],
                    in_=znd[2048 * h : 2048 * (h + 1), k * P : (k + 1) * P],
                    transpose=True,
                )
            # one-ahead front for the slab after next
            if h + 2 < NU:
                front(zu[h + 2], TPU * (h + 2), TPU)
            # matmuls + exp row-sums for all row tiles against this block
            for t in range(RT):
                ps = ps_pool.tile([P, CHUNK], f32, tag="ps", name="ps")
                for k in range(2):
                    for q in range(4):
                        nc.tensor.matmul(
                            ps[:, q * 512 : (q + 1) * 512],
                            znrT[k][:, t * P : (t + 1) * P],
                            znT[k][:, h * CHUNK + q * 512 : h * CHUNK + (q + 1) * 512],
                            start=(k == 0),
                            stop=(k == 1),
                            skip_group_check=True,
                        )
                es = scr_pool.tile([P, CHUNK], bf16, tag="escr", name="es")
                idx = HS * t + h
                nc.scalar.activation(
                    es[:],
                    ps[:],
                    Exp,
                    bias=bias_m2[:],
                    scale=2.0,
                    accum_out=sums[:, idx : idx + 1],
                )

        # ---- lse - pos = ln(S - 1) + 2 - pos
        nc.vector.tensor_reduce(
            out=Ssum[:],
            in_=sums[:].rearrange("p (t h) -> p t h", h=HS),
            axis=AxX,
            op=ADD,
        )
        nc.vector.tensor_scalar_add(Ssum[:], Ssum[:], -1.0)
        nc.scalar.activation(lnS[:], Ssum[:], Ln)
        nc.vector.tensor_sub(outsb[:], lnS[:], posM[:])
        nc.vector.tensor_scalar_add(outsb[:], outsb[:], 2.0)
        nc.sync.dma_start(out=out[:], in_=outsb[:])

    nc.compile()
    return nc


def _install_ntff_hook():
    """Provide antenv.axon_hooks (absent in this image) so trace=True can
    capture NTFF profiles via libaxon_pjrt's C ABI."""
    import sys, types, ctypes, contextlib

    try:
        from antenv.axon_hooks import get_axon_ntff_profile_hook  # noqa: F401

        return True
    except ImportError:
        pass
    try:
        import antenv

        lib = ctypes.CDLL("/opt/axon/libaxon_pjrt.so")
        if not hasattr(lib, "axon_start_nrt_profile"):
            return False
        lib.axon_start_nrt_profile.argtypes = [
            ctypes.POINTER(ctypes.c_int64),
            ctypes.c_size_t,
        ]
        lib.axon_start_nrt_profile.restype = ctypes.c_int64
        lib.axon_stop_nrt_profile.argtypes = [ctypes.c_char_p]
        lib.axon_stop_nrt_profile.restype = ctypes.c_int64

        @contextlib.contextmanager
        def _hook(output_dir, device_ids):
            import jax

            jax.devices()
            if device_ids:
                ids = (ctypes.c_int64 * len(device_ids))(*device_ids)
                rc = lib.axon_start_nrt_profile(ids, len(device_ids))
            else:
                rc = lib.axon_start_nrt_profile(None, 0)
            if rc != 0:
                raise RuntimeError(f"axon_start_nrt_profile rc={rc}")
            try:
                yield
            finally:
                n = lib.axon_stop_nrt_profile(str(output_dir).encode())
                print(f"ntff profile: {n} file(s) written to {output_dir}")

        mod = types.ModuleType("antenv.axon_hooks")
        _state = {"hook": _hook}
        mod.set_axon_ntff_profile_hook = lambda h: _state.__setitem__("hook", h)
        mod.get_axon_ntff_profile_hook = lambda: _state["hook"]
        sys.modules["antenv.axon_hooks"] = mod
        antenv.axon_hooks = mod
        return True
    except Exception as e:
        print(f"ntff hook install failed: {e}")
        return False


def _get_nc():
    if "nc" not in _cache:
        _cache["nc"] = _build_nc()
    return _cache["nc"]


def kernel(z_i, z_j):
    global LAST_EXEC_TIME_NS, LAST_RESULTS
    from concourse.bass_utils import run_bass_kernel_spmd

    z = np.ascontiguousarray(
        np.concatenate([np.asarray(z_i), np.asarray(z_j)], axis=0), dtype=np.float32
    )
    in_maps = []
    for k in range(NCORES):
        lo = RPC * k
        plo = (lo + B) % N
        in_maps.append(
            {
                "zall": z,
                "zrows": np.ascontiguousarray(z[lo : lo + RPC]),
                "zpart": np.ascontiguousarray(z[plo : plo + RPC]),
            }
        )

    nc = _get_nc()
    trace = os.environ.get("BASS_KERNEL_TRACE", "0") == "1"
    if trace:
        trace = _install_ntff_hook()
    res = run_bass_kernel_spmd(nc, in_maps, core_ids=list(range(NCORES)), trace=trace)
    LAST_RESULTS = res
    LAST_EXEC_TIME_NS = res.exec_time_ns

    total = 0.0
    for k in range(NCORES):
        total += float(np.sum(np.asarray(res.results[k]["out"], dtype=np.float64)))
    return np.array(total / N, dtype=np.float32)
